# revision 1
# baseline (speedup 1.0000x reference)
"""Trainium2 Bass kernel for a 2-layer GraphSAGE (sum aggregation) GNN.

Strategy (8 NeuronCores, SPMD, two launches):
  - Nodes (dst) sharded 12500/core. Edges partitioned by dst owner.
  - Per core, dst nodes are sorted by in-degree (descending) into "ranks";
    ranks tile into 98 blocks of 128 (12544 slots, 44 zero pads).
  - Launch 1: layer-1 aggregation via round-structured [128,1]-indexed
    indirect DMA gathers of 512B x rows, DVE-accumulated into SBUF agg;
    then per tile h = relu(agg @ Wn1 + x @ Ws1 + b1) on PE, and the
    8-wide projections z = h @ Wn2 and o2 = h @ Ws2 + b2.
  - Using segsum(h[src]) @ Wn2 == segsum((h @ Wn2)[src]), only z (8 wide)
    must be exchanged across cores. The host concatenates the per-core z
    shards (collectives are unavailable on this runtime path).
  - Launch 2: layer-2 aggregation gathers 32B z rows with the same round
    structure, adds o2, applies log_softmax; host inverse-permutes rows.

The host side only reshuffles indices / rows (graph partitioning and the
z-shard concat); all feature compute happens on device.
"""

import sys

import numpy as np

sys.path.insert(0, "/opt/trn_rl_repo")

import concourse.bass as bass
import concourse.mybir as mybir
import concourse.tile as tile
from concourse import bacc
from concourse.bass_utils import run_bass_kernel_spmd
from concourse.masks import make_identity

P = 128
N_NODES = 100000
N_CORES = 8
NPC = N_NODES // N_CORES  # 12500
NT = 98  # rank tiles per core
NR = NT * P  # 12544 rank slots per core
NCLS = 8
ZROW_X = N_NODES  # zeros row appended to x gather table
ZROW_Z = NPC  # core-0 pad rank (z value is exactly 0 by construction)
F32 = mybir.dt.float32
I32 = mybir.dt.int32


def _prep_host(x, edge_src, edge_dst):
    """Partition edges by dst core, degree-sort dst ranks, build round-major
    per-tile gather index arrays. Returns per-core arrays + globals."""
    edge_src = np.asarray(edge_src)
    edge_dst = np.asarray(edge_dst)
    core_of = edge_dst // NPC

    orders = []  # per core: rank -> local dst id
    deg_sorted = []  # per core: degree per rank (desc)
    per_core = []
    for k in range(N_CORES):
        m = core_of == k
        s = edge_src[m]
        dl = edge_dst[m] - k * NPC
        deg = np.bincount(dl, minlength=NPC)
        order = np.argsort(-deg, kind="stable")
        rank_of = np.empty(NPC, dtype=np.int64)
        rank_of[order] = np.arange(NPC)
        orders.append(order)
        deg_sorted.append(deg[order])
        per_core.append((s, rank_of[dl]))

    # global z position of each original node (layout of concatenated z)
    zpos = np.empty(N_NODES, dtype=np.int64)
    for k in range(N_CORES):
        zpos[k * NPC + orders[k]] = k * NR + np.arange(NPC)

    # global per-tile round counts (max over cores; >=1)
    R = np.ones(NT, dtype=np.int64)
    for k in range(N_CORES):
        for t in range(NT):
            lead = t * P
            if lead < NPC:
                R[t] = max(R[t], deg_sorted[k][lead])
    off = np.zeros(NT + 1, dtype=np.int64)
    off[1:] = np.cumsum(R)
    TK = int(off[-1])

    I1s, I2s, xTs = [], [], []
    for k in range(N_CORES):
        s, ranks = per_core[k]
        eo = np.argsort(ranks, kind="stable")
        rs = ranks[eo]
        ss = s[eo]
        starts = np.searchsorted(rs, np.arange(NPC))
        occ = np.arange(len(rs)) - starts[rs]
        maxR = int(R.max())
        A1 = np.full((NR, maxR), ZROW_X, dtype=np.int32)
        A2 = np.full((NR, maxR), ZROW_Z, dtype=np.int32)
        A1[rs, occ] = ss
        A2[rs, occ] = zpos[ss]
        I1 = np.empty((P, TK), dtype=np.int32)
        I2 = np.empty((P, TK), dtype=np.int32)
        for t in range(NT):
            blk = slice(t * P, (t + 1) * P)
            I1[:, off[t] : off[t + 1]] = A1[blk, : R[t]]
            I2[:, off[t] : off[t + 1]] = A2[blk, : R[t]]
        I1s.append(I1)
        I2s.append(I2)
        xT = np.zeros((P, NR), dtype=np.float32)
        xT[:, :NPC] = x[k * NPC + orders[k]].T
        xTs.append(np.ascontiguousarray(xT))

    return orders, R, off, TK, I1s, I2s, xTs


def _build_nc1(R, off, TK):
    """Launch 1: layer-1 aggregate + matmuls; outputs z and o2 per core."""
    nc = bacc.Bacc(
        "TRN2", target_bir_lowering=False, debug=False, num_devices=N_CORES
    )
    xg = nc.dram_tensor("xg", [N_NODES + 1, P], F32, kind="ExternalInput").ap()
    xT = nc.dram_tensor("xT", [P, NR], F32, kind="ExternalInput").ap()
    I1 = nc.dram_tensor("I1", [P, TK], I32, kind="ExternalInput").ap()
    W1n = nc.dram_tensor("W1n", [P, P], F32, kind="ExternalInput").ap()
    W1s = nc.dram_tensor("W1s", [P, P], F32, kind="ExternalInput").ap()
    W2n = nc.dram_tensor("W2n", [P, NCLS], F32, kind="ExternalInput").ap()
    W2s = nc.dram_tensor("W2s", [P, NCLS], F32, kind="ExternalInput").ap()
    b1 = nc.dram_tensor("b1", [1, P], F32, kind="ExternalInput").ap()
    b2 = nc.dram_tensor("b2", [1, NCLS], F32, kind="ExternalInput").ap()
    z_k = nc.dram_tensor("z", [P, NT * NCLS], F32, kind="ExternalOutput").ap()
    o2_k = nc.dram_tensor("o2", [P, NT * NCLS], F32, kind="ExternalOutput").ap()

    with tile.TileContext(nc) as tc:
        with (
            tc.tile_pool(name="persist", bufs=1) as pp,
            tc.tile_pool(name="gather", bufs=8) as gp,
            tc.tile_pool(name="work", bufs=3) as wp,
            tc.tile_pool(name="psum", bufs=1, space="PSUM") as psp,
        ):
            w1n = pp.tile([P, P], F32, tag="w1n")
            w1s = pp.tile([P, P], F32, tag="w1s")
            w2n = pp.tile([P, NCLS], F32, tag="w2n")
            w2s = pp.tile([P, NCLS], F32, tag="w2s")
            b1t = pp.tile([1, P], F32, tag="b1")
            b2t = pp.tile([1, NCLS], F32, tag="b2")
            ones = pp.tile([1, P], F32, tag="ones")
            ident = pp.tile([P, P], F32, tag="ident")
            i1t = pp.tile([P, TK], I32, tag="i1")
            xTt = pp.tile([P, NR], F32, tag="xT")
            agg = pp.tile([P, NR], F32, tag="agg")
            zsb = pp.tile([P, NT * NCLS], F32, tag="z")
            o2sb = pp.tile([P, NT * NCLS], F32, tag="o2")

            nc.sync.dma_start(out=w1n[:], in_=W1n[:])
            nc.sync.dma_start(out=w1s[:], in_=W1s[:])
            nc.sync.dma_start(out=w2n[:], in_=W2n[:])
            nc.sync.dma_start(out=w2s[:], in_=W2s[:])
            nc.sync.dma_start(out=b1t[:], in_=b1[:])
            nc.sync.dma_start(out=b2t[:], in_=b2[:])
            nc.sync.dma_start(out=i1t[:], in_=I1[:])
            nc.sync.dma_start(out=xTt[:], in_=xT[:])
            nc.vector.memset(ones[:], 1.0)
            make_identity(nc, ident[:])

            # layer-1 gather+accumulate (tile-major rounds)
            for t in range(NT):
                csl = slice(t * P, (t + 1) * P)
                for r in range(int(R[t])):
                    col = int(off[t]) + r
                    buf = gp.tile([P, P], F32, tag="g1")
                    nc.gpsimd.indirect_dma_start(
                        out=buf[:],
                        out_offset=None,
                        in_=xg[:],
                        in_offset=bass.IndirectOffsetOnAxis(
                            ap=i1t[:, col : col + 1], axis=0
                        ),
                    )
                    if r == 0:
                        nc.vector.tensor_copy(out=agg[:, csl], in_=buf[:])
                    else:
                        nc.vector.tensor_add(
                            out=agg[:, csl], in0=agg[:, csl], in1=buf[:]
                        )

            # per-tile matmuls: h, z, self-path of layer 2
            for t in range(NT):
                csl = slice(t * P, (t + 1) * P)
                zsl = slice(t * NCLS, (t + 1) * NCLS)
                aggT_ps = psp.tile([P, P], F32, tag="aggT_ps")
                nc.tensor.transpose(
                    out=aggT_ps[:], in_=agg[:, csl], identity=ident[:]
                )
                aggT = wp.tile([P, P], F32, tag="aggT")
                nc.vector.tensor_copy(out=aggT[:], in_=aggT_ps[:])
                h_ps = psp.tile([P, P], F32, tag="h_ps")
                nc.tensor.matmul(
                    out=h_ps[:], lhsT=aggT[:], rhs=w1n[:], start=True, stop=False
                )
                nc.tensor.matmul(
                    out=h_ps[:], lhsT=xTt[:, csl], rhs=w1s[:],
                    start=False, stop=False,
                )
                nc.tensor.matmul(
                    out=h_ps[:], lhsT=ones[:1, :], rhs=b1t[:1, :],
                    start=False, stop=True,
                )
                h = wp.tile([P, P], F32, tag="h")
                nc.scalar.activation(
                    out=h[:], in_=h_ps[:], func=mybir.ActivationFunctionType.Relu
                )
                hT_ps = psp.tile([P, P], F32, tag="hT_ps")
                nc.tensor.transpose(out=hT_ps[:], in_=h[:], identity=ident[:])
                hT = wp.tile([P, P], F32, tag="hT")
                nc.vector.tensor_copy(out=hT[:], in_=hT_ps[:])
                z_ps = psp.tile([P, NCLS], F32, tag="z_ps")
                nc.tensor.matmul(
                    out=z_ps[:], lhsT=hT[:], rhs=w2n[:], start=True, stop=True
                )
                nc.vector.tensor_copy(out=zsb[:, zsl], in_=z_ps[:])
                o2_ps = psp.tile([P, NCLS], F32, tag="o2_ps")
                nc.tensor.matmul(
                    out=o2_ps[:], lhsT=hT[:], rhs=w2s[:], start=True, stop=False
                )
                nc.tensor.matmul(
                    out=o2_ps[:], lhsT=ones[:1, :], rhs=b2t[:1, :],
                    start=False, stop=True,
                )
                nc.vector.tensor_copy(out=o2sb[:, zsl], in_=o2_ps[:])

            nc.sync.dma_start(out=z_k, in_=zsb[:])
            nc.sync.dma_start(out=o2_k, in_=o2sb[:])

    nc.compile()
    return nc


def _build_nc2(R, off, TK):
    """Launch 2: layer-2 gather of z rows, add self-path, log_softmax."""
    nc = bacc.Bacc(
        "TRN2", target_bir_lowering=False, debug=False, num_devices=N_CORES
    )
    zf = nc.dram_tensor(
        "zf", [N_CORES * NR, NCLS], F32, kind="ExternalInput"
    ).ap()
    o2_k = nc.dram_tensor("o2", [P, NT * NCLS], F32, kind="ExternalInput").ap()
    I2 = nc.dram_tensor("I2", [P, TK], I32, kind="ExternalInput").ap()
    out = nc.dram_tensor("out", [P, NT * NCLS], F32, kind="ExternalOutput").ap()

    with tile.TileContext(nc) as tc:
        with (
            tc.tile_pool(name="persist", bufs=1) as pp,
            tc.tile_pool(name="gather", bufs=8) as gp,
        ):
            i2t = pp.tile([P, TK], I32, tag="i2")
            o2sb = pp.tile([P, NT * NCLS], F32, tag="o2")
            a2sb = pp.tile([P, NT * NCLS], F32, tag="a2")
            nc.sync.dma_start(out=i2t[:], in_=I2[:])
            nc.sync.dma_start(out=o2sb[:], in_=o2_k[:])

            for t in range(NT):
                zsl = slice(t * NCLS, (t + 1) * NCLS)
                for r in range(int(R[t])):
                    col = int(off[t]) + r
                    buf2 = gp.tile([P, NCLS], F32, tag="g2")
                    nc.gpsimd.indirect_dma_start(
                        out=buf2[:],
                        out_offset=None,
                        in_=zf[:],
                        in_offset=bass.IndirectOffsetOnAxis(
                            ap=i2t[:, col : col + 1], axis=0
                        ),
                    )
                    if r == 0:
                        nc.vector.tensor_copy(out=a2sb[:, zsl], in_=buf2[:])
                    else:
                        nc.vector.tensor_add(
                            out=a2sb[:, zsl], in0=a2sb[:, zsl], in1=buf2[:]
                        )

            nc.vector.tensor_add(out=a2sb[:], in0=a2sb[:], in1=o2sb[:])
            a3 = a2sb[:].rearrange("p (t c) -> p t c", c=NCLS)
            mx = pp.tile([P, NT], F32, tag="mx")
            nc.vector.tensor_reduce(
                out=mx[:], in_=a3, axis=mybir.AxisListType.X,
                op=mybir.AluOpType.max,
            )
            mxb = mx[:].unsqueeze(2).to_broadcast([P, NT, NCLS])
            nc.vector.tensor_tensor(
                out=a3, in0=a3, in1=mxb, op=mybir.AluOpType.subtract
            )
            ex = pp.tile([P, NT * NCLS], F32, tag="ex")
            nc.scalar.activation(
                out=ex[:], in_=a2sb[:], func=mybir.ActivationFunctionType.Exp
            )
            sm = pp.tile([P, NT], F32, tag="sm")
            nc.vector.tensor_reduce(
                out=sm[:],
                in_=ex[:].rearrange("p (t c) -> p t c", c=NCLS),
                axis=mybir.AxisListType.X,
                op=mybir.AluOpType.add,
            )
            lg = pp.tile([P, NT], F32, tag="lg")
            nc.scalar.activation(
                out=lg[:], in_=sm[:], func=mybir.ActivationFunctionType.Ln
            )
            lgb = lg[:].unsqueeze(2).to_broadcast([P, NT, NCLS])
            nc.vector.tensor_tensor(
                out=a3, in0=a3, in1=lgb, op=mybir.AluOpType.subtract
            )
            nc.sync.dma_start(out=out[:], in_=a2sb[:])

    nc.compile()
    return nc


def kernel(
    x, edge_src, edge_dst, W_neigh1, W_self1, b1, W_neigh2, W_self2, b2
):
    x = np.ascontiguousarray(np.asarray(x, dtype=np.float32))
    orders, R, off, TK, I1s, I2s, xTs = _prep_host(x, edge_src, edge_dst)

    xg = np.vstack([x, np.zeros((1, P), np.float32)])
    common = {
        "xg": xg,
        "W1n": np.asarray(W_neigh1, np.float32),
        "W1s": np.asarray(W_self1, np.float32),
        "W2n": np.asarray(W_neigh2, np.float32),
        "W2s": np.asarray(W_self2, np.float32),
        "b1": np.asarray(b1, np.float32).reshape(1, P),
        "b2": np.asarray(b2, np.float32).reshape(1, NCLS),
    }
    in_maps1 = [
        {**common, "xT": xTs[k], "I1": I1s[k]} for k in range(N_CORES)
    ]

    nc1 = _build_nc1(R, off, TK)
    res1 = run_bass_kernel_spmd(nc1, in_maps1, list(range(N_CORES)))

    def _rows(a):  # [P, NT*NCLS] sbuf layout -> [NR, NCLS] rank rows
        return np.ascontiguousarray(
            a.reshape(P, NT, NCLS).transpose(1, 0, 2).reshape(NR, NCLS)
        )

    z_full = np.concatenate(
        [_rows(res1.results[k]["z"]) for k in range(N_CORES)], axis=0
    )
    in_maps2 = [
        {"zf": z_full, "o2": res1.results[k]["o2"], "I2": I2s[k]}
        for k in range(N_CORES)
    ]
    nc2 = _build_nc2(R, off, TK)
    res2 = run_bass_kernel_spmd(nc2, in_maps2, list(range(N_CORES)))

    out_full = np.empty((N_NODES, NCLS), dtype=np.float32)
    for k in range(N_CORES):
        out_full[k * NPC + orders[k]] = _rows(res2.results[k]["out"])[:NPC]
    return out_full


if __name__ == "__main__":
    import jax

    import reference

    cpu = jax.devices("cpu")[0]
    with jax.default_device(cpu):
        inputs = {k: np.asarray(v) for k, v in reference.setup_inputs().items()}
        exp = np.asarray(
            reference.reference(**{k: jax.device_put(v, cpu) for k, v in inputs.items()})
        )
    got = kernel(**inputs)
    err = np.abs(got - exp)
    rel = err / (np.abs(exp) + 1e-6)
    print("max abs err:", err.max(), "max rel err:", rel.max())



# revision 2
# speedup vs baseline: 1.2498x; 1.2498x over previous
"""Trainium2 Bass kernel for a 2-layer GraphSAGE (sum aggregation) GNN — v2.

Strategy (8 NeuronCores, SPMD, two launches):
  - Nodes (dst) sharded 12500/core; dsts degree-sorted into 98 tiles of 128
    "ranks". Edges land in per-(tile, group) slot grids: slot (p, c) holds
    the c-th source of rank p, padded with -1 (dma_gather writes zeros for
    mid-list negative idxs — the additive identity).
  - Gathers use nc.gpsimd.dma_gather (int16 idxs, purpose-built Q7 ucode,
    load_library(mlp)) instead of per-round indirect_dma_start: many rounds
    per instruction instead of 1, cutting Pool-engine instruction count ~4-8x.
  - int16 idx range (<=32767) forces table splits: x into 4 quarters of
    25000 rows (layer 1); z into 4 owner-pair tables of 25088 rows padded to
    64 f32 = 256B rows (dma_gather min row size; layer 2).
  - Per tile: DVE fold (halving adds) reduces the gathered slots to the
    aggregated row, then PE: h = relu(aggT@W1n + xT@W1s + b1),
    z = h@W2n, o2 = h@W2s + b2. Only z (8 wide) crosses cores (host concat).
  - Launch 2: same grid structure gathers z rows (256B padded), strided
    8-wide folds, + o2, log_softmax.
"""

import sys

import numpy as np

sys.path.insert(0, "/opt/trn_rl_repo")

import concourse.bass as bass
import concourse.mybir as mybir
import concourse.tile as tile
from concourse import bacc, library_config
from concourse.bass_utils import run_bass_kernel_spmd

P = 128
N_NODES = 100000
N_CORES = 8
NPC = N_NODES // N_CORES  # 12500
NT = 98  # rank tiles per core
NR = NT * P  # 12544 rank slots per core
NCLS = 8
G1 = 4  # x table split (25000 rows each, int16-addressable)
Q1 = N_NODES // G1  # 25000
G2 = 4  # z table split (owner pairs: 25088 rows each)
Q2 = 2 * NR  # 25088
ZPAD = 64  # padded z row elems (256B = dma_gather minimum)
RC = 8  # rounds (128-row groups) per dma_gather instruction (<=1024 idxs)
F32 = mybir.dt.float32
I16 = mybir.dt.int16


def _build_grid(r, g, v, G, zrow):
    """Slot grids for one core. r: dst rank/edge; g: table group/edge;
    v: local table row/edge; zrow: zeros-row index used for pad slots.
    Returns (A [TOTW, 128] int16 slot matrix, R [NT, G] rounds,
    col_base [NT, G], tile_base [NT+1])."""
    o = np.lexsort((r, g))
    gs, rs, vs = g[o], r[o], v[o]
    gr = gs.astype(np.int64) * NR + rs
    occ = np.arange(len(gr)) - np.searchsorted(gr, gr)
    ts = rs // P
    ps = rs % P
    R = np.zeros((NT, G), np.int64)
    np.maximum.at(R, (ts, gs), occ + 1)
    col_base = np.zeros((NT, G), np.int64)
    col_base[:, 1:] = np.cumsum(R, axis=1)[:, :-1]
    W = R.sum(axis=1)  # rounds per tile
    tile_base = np.zeros(NT + 1, np.int64)
    tile_base[1:] = np.cumsum(W)
    TOTW = int(tile_base[-1])
    A = np.full((TOTW, P), zrow, np.int16)
    gcol = tile_base[ts] + col_base[ts, gs] + occ
    A[gcol, ps] = vs.astype(np.int16)
    return A, R, col_base, tile_base


def _profile_order(dl, g, G):
    """Order local dst nodes so tiles hold similar per-group-count profiles
    (minimizes per-(tile,group) max-padding). Descending lexsort by profile."""
    cnt = np.zeros((NPC, G), np.int64)
    np.add.at(cnt, (dl, g), 1)
    order = np.lexsort(tuple(cnt[:, i] for i in range(G - 1, -1, -1)))[::-1]
    rank_of = np.empty(NPC, np.int64)
    rank_of[order] = np.arange(NPC)
    return order, rank_of


def _prep_host(x, edge_src, edge_dst):
    edge_src = np.asarray(edge_src).astype(np.int64)
    edge_dst = np.asarray(edge_dst).astype(np.int64)
    core_of = edge_dst // NPC
    S, D = [], []
    for k in range(N_CORES):
        m = core_of == k
        S.append(edge_src[m])
        D.append(edge_dst[m] - k * NPC)

    # pass A: layer-1 orders (by x-quarter profile) -> zpos layout
    orders1, grids1 = [], []
    rank1_of = np.empty(N_NODES, np.int64)
    for k in range(N_CORES):
        order1, r1 = _profile_order(D[k], S[k] // Q1, G1)
        orders1.append(order1)
        rank1_of[k * NPC : (k + 1) * NPC] = r1
        grids1.append(_build_grid(r1[D[k]], S[k] // Q1, S[k] % Q1, G1, Q1))
    owner = np.arange(N_NODES) // NPC
    zpos = owner * NR + rank1_of

    # pass B: layer-2 orders (by z-group profile)
    orders2, grids2 = [], []
    for k in range(N_CORES):
        zp = zpos[S[k]]
        order2, r2 = _profile_order(D[k], zp // Q2, G2)
        orders2.append(order2)
        grids2.append(_build_grid(r2[D[k]], zp // Q2, zp % Q2, G2, Q2))

    xTs = []
    x = np.ascontiguousarray(np.asarray(x, dtype=np.float32))
    for k in range(N_CORES):
        xT = np.zeros((P, NR), np.float32)
        xT[:, :NPC] = x[k * NPC + orders1[k]].T
        xTs.append(np.ascontiguousarray(xT))

    return orders1, orders2, grids1, grids2, xTs


def _chunks(n):
    """Split n rounds into chunks of <= RC."""
    out = []
    a = 0
    while a < n:
        b = min(a + RC, n)
        out.append((a, b))
        a = b
    return out


def _emit_gathers(nc, grid, tabs, it, buf, t, elem):
    """Emit dma_gather instructions for tile t into buf; returns rounds W."""
    _, R, col_base, tile_base = grid
    W = int(tile_base[t + 1] - tile_base[t])
    for g in range(len(tabs)):
        Rg = int(R[t, g])
        if Rg == 0:
            continue
        lc = int(col_base[t, g])  # local col (rounds) within tile
        gc = int(tile_base[t]) + lc  # global col
        for a, b in _chunks(Rg):
            nb = b - a
            nc.gpsimd.dma_gather(
                out_ap=buf[:, (lc + a) * elem : (lc + b) * elem].rearrange(
                    "p (c e) -> p c e", e=elem
                ),
                in_ap=tabs[g],
                idxs_ap=it[:, (gc + a) * 8 : (gc + b) * 8],
                num_idxs=nb * P,
                num_idxs_reg=nb * P,
                elem_size=elem,
                single_packet=True,
            )
    return W


def _build_nc1(grid1, reps=1):
    """Launch 1: layer-1 aggregate + matmuls; outputs z and o2."""
    idx1, R1, col_base1, tile_base1 = grid1
    TOTW = int(tile_base1[-1])
    WMAX = int((tile_base1[1:] - tile_base1[:-1]).max())
    nc = bacc.Bacc(
        "TRN2", target_bir_lowering=False, debug=False, num_devices=N_CORES
    )
    xq = [
        nc.dram_tensor(f"xq{g}", [Q1 + 1, P], F32, kind="ExternalInput").ap()
        for g in range(G1)
    ]
    xT = nc.dram_tensor("xT", [P, NR], F32, kind="ExternalInput").ap()
    I1 = nc.dram_tensor("I1", [P, TOTW * 8], I16, kind="ExternalInput").ap()
    W1n = nc.dram_tensor("W1n", [P, P], F32, kind="ExternalInput").ap()
    W1s = nc.dram_tensor("W1s", [P, P], F32, kind="ExternalInput").ap()
    W2n = nc.dram_tensor("W2n", [P, NCLS], F32, kind="ExternalInput").ap()
    W2s = nc.dram_tensor("W2s", [P, NCLS], F32, kind="ExternalInput").ap()
    b1 = nc.dram_tensor("b1", [1, P], F32, kind="ExternalInput").ap()
    b2 = nc.dram_tensor("b2", [1, NCLS], F32, kind="ExternalInput").ap()
    Ident = nc.dram_tensor("Ident", [P, P], F32, kind="ExternalInput").ap()
    z_k = nc.dram_tensor("z", [P, NT * NCLS], F32, kind="ExternalOutput").ap()
    o2_k = nc.dram_tensor("o2", [P, NT * NCLS], F32, kind="ExternalOutput").ap()

    with tile.TileContext(nc) as tc:
        with (
            tc.tile_pool(name="persist", bufs=1) as pp,
            tc.tile_pool(name="gather", bufs=2) as gp,
            tc.tile_pool(name="work", bufs=3) as wp,
            tc.tile_pool(name="psum", bufs=1, space="PSUM") as psp,
        ):
            w1n = pp.tile([P, P], F32, tag="w1n")
            w1s = pp.tile([P, P], F32, tag="w1s")
            w2n = pp.tile([P, NCLS], F32, tag="w2n")
            w2s = pp.tile([P, NCLS], F32, tag="w2s")
            b1t = pp.tile([1, P], F32, tag="b1")
            b2t = pp.tile([1, NCLS], F32, tag="b2")
            ones = pp.tile([1, P], F32, tag="ones")
            ident = pp.tile([P, P], F32, tag="ident")
            i1t = pp.tile([P, TOTW * 8], I16, tag="i1")
            xTt = pp.tile([P, NR], F32, tag="xT")
            zsb = pp.tile([P, NT * NCLS], F32, tag="z")
            o2sb = pp.tile([P, NT * NCLS], F32, tag="o2")

            nc.gpsimd.load_library(library_config.mlp)
            nc.sync.dma_start(out=w1n[:], in_=W1n[:])
            nc.sync.dma_start(out=w1s[:], in_=W1s[:])
            nc.sync.dma_start(out=w2n[:], in_=W2n[:])
            nc.sync.dma_start(out=w2s[:], in_=W2s[:])
            nc.sync.dma_start(out=b1t[:], in_=b1[:])
            nc.sync.dma_start(out=b2t[:], in_=b2[:])
            nc.sync.dma_start(out=i1t[:], in_=I1[:])
            nc.sync.dma_start(out=xTt[:], in_=xT[:])
            nc.sync.dma_start(out=ident[:], in_=Ident[:])
            nc.vector.memset(ones[:], 1.0)

            for rep in range(reps):
                for t in range(NT):
                    csl = slice(t * P, (t + 1) * P)
                    zsl = slice(t * NCLS, (t + 1) * NCLS)
                    buf = gp.tile([P, WMAX * P], F32, tag="g1")
                    W = _emit_gathers(
                        nc, grid1, xq, i1t[:], buf[:], t, P
                    )
                    if W == 0:
                        nc.vector.memset(buf[:, :P], 0.0)
                    w = W
                    while w > 1:
                        h = w // 2
                        nc.vector.tensor_add(
                            out=buf[:, : h * P],
                            in0=buf[:, : h * P],
                            in1=buf[:, (w - h) * P : w * P],
                        )
                        w -= h
                    aggT_ps = psp.tile([P, P], F32, tag="aggT_ps")
                    nc.tensor.transpose(
                        out=aggT_ps[:], in_=buf[:, :P], identity=ident[:]
                    )
                    aggT = wp.tile([P, P], F32, tag="aggT")
                    nc.vector.tensor_copy(out=aggT[:], in_=aggT_ps[:])
                    h_ps = psp.tile([P, P], F32, tag="h_ps")
                    nc.tensor.matmul(
                        out=h_ps[:], lhsT=aggT[:], rhs=w1n[:],
                        start=True, stop=False,
                    )
                    nc.tensor.matmul(
                        out=h_ps[:], lhsT=xTt[:, csl], rhs=w1s[:],
                        start=False, stop=False,
                    )
                    nc.tensor.matmul(
                        out=h_ps[:], lhsT=ones[:1, :], rhs=b1t[:1, :],
                        start=False, stop=True,
                    )
                    hsb = wp.tile([P, P], F32, tag="h")
                    nc.scalar.activation(
                        out=hsb[:], in_=h_ps[:],
                        func=mybir.ActivationFunctionType.Relu,
                    )
                    hT_ps = psp.tile([P, P], F32, tag="hT_ps")
                    nc.tensor.transpose(
                        out=hT_ps[:], in_=hsb[:], identity=ident[:]
                    )
                    hT = wp.tile([P, P], F32, tag="hT")
                    nc.vector.tensor_copy(out=hT[:], in_=hT_ps[:])
                    z_ps = psp.tile([P, NCLS], F32, tag="z_ps")
                    nc.tensor.matmul(
                        out=z_ps[:], lhsT=hT[:], rhs=w2n[:],
                        start=True, stop=True,
                    )
                    nc.vector.tensor_copy(out=zsb[:, zsl], in_=z_ps[:])
                    o2_ps = psp.tile([P, NCLS], F32, tag="o2_ps")
                    nc.tensor.matmul(
                        out=o2_ps[:], lhsT=hT[:], rhs=w2s[:],
                        start=True, stop=False,
                    )
                    nc.tensor.matmul(
                        out=o2_ps[:], lhsT=ones[:1, :], rhs=b2t[:1, :],
                        start=False, stop=True,
                    )
                    nc.vector.tensor_copy(out=o2sb[:, zsl], in_=o2_ps[:])

            nc.sync.dma_start(out=z_k, in_=zsb[:])
            nc.sync.dma_start(out=o2_k, in_=o2sb[:])

    nc.compile()
    return nc


def _build_nc2(grid2, reps=1):
    """Launch 2: layer-2 gather of padded z rows, + self path, log_softmax."""
    idx2, R2, col_base2, tile_base2 = grid2
    TOTW = int(tile_base2[-1])
    WMAX = int((tile_base2[1:] - tile_base2[:-1]).max())
    nc = bacc.Bacc(
        "TRN2", target_bir_lowering=False, debug=False, num_devices=N_CORES
    )
    zq = [
        nc.dram_tensor(f"zq{g}", [Q2 + 1, ZPAD], F32, kind="ExternalInput").ap()
        for g in range(G2)
    ]
    o2_k = nc.dram_tensor("o2", [P, NT * NCLS], F32, kind="ExternalInput").ap()
    I2 = nc.dram_tensor("I2", [P, TOTW * 8], I16, kind="ExternalInput").ap()
    out = nc.dram_tensor("out", [P, NT * NCLS], F32, kind="ExternalOutput").ap()

    with tile.TileContext(nc) as tc:
        with (
            tc.tile_pool(name="persist", bufs=1) as pp,
            tc.tile_pool(name="gather", bufs=2) as gp,
        ):
            i2t = pp.tile([P, TOTW * 8], I16, tag="i2")
            o2sb = pp.tile([P, NT * NCLS], F32, tag="o2")
            a2sb = pp.tile([P, NT * NCLS], F32, tag="a2")
            nc.gpsimd.load_library(library_config.mlp)
            nc.sync.dma_start(out=i2t[:], in_=I2[:])
            nc.sync.dma_start(out=o2sb[:], in_=o2_k[:])

            for rep in range(reps):
                for t in range(NT):
                    zsl = slice(t * NCLS, (t + 1) * NCLS)
                    buf = gp.tile([P, WMAX * ZPAD], F32, tag="g2")
                    W = _emit_gathers(
                        nc, grid2, zq, i2t[:], buf[:], t, ZPAD
                    )
                    if W == 0:
                        nc.vector.memset(buf[:, :ZPAD], 0.0)
                    v3 = buf[:].rearrange("p (w e) -> p w e", e=ZPAD)
                    w = W
                    while w > 1:
                        h = w // 2
                        nc.vector.tensor_tensor(
                            out=v3[:, :h, :NCLS],
                            in0=v3[:, :h, :NCLS],
                            in1=v3[:, w - h : w, :NCLS],
                            op=mybir.AluOpType.add,
                        )
                        w -= h
                    nc.vector.tensor_add(
                        out=a2sb[:, zsl], in0=buf[:, :NCLS], in1=o2sb[:, zsl]
                    )

            a3 = a2sb[:].rearrange("p (t c) -> p t c", c=NCLS)
            mx = pp.tile([P, NT], F32, tag="mx")
            nc.vector.tensor_reduce(
                out=mx[:], in_=a3, axis=mybir.AxisListType.X,
                op=mybir.AluOpType.max,
            )
            mxb = mx[:].unsqueeze(2).to_broadcast([P, NT, NCLS])
            nc.vector.tensor_tensor(
                out=a3, in0=a3, in1=mxb, op=mybir.AluOpType.subtract
            )
            ex = pp.tile([P, NT * NCLS], F32, tag="ex")
            nc.scalar.activation(
                out=ex[:], in_=a2sb[:], func=mybir.ActivationFunctionType.Exp
            )
            sm = pp.tile([P, NT], F32, tag="sm")
            nc.vector.tensor_reduce(
                out=sm[:],
                in_=ex[:].rearrange("p (t c) -> p t c", c=NCLS),
                axis=mybir.AxisListType.X,
                op=mybir.AluOpType.add,
            )
            lg = pp.tile([P, NT], F32, tag="lg")
            nc.scalar.activation(
                out=lg[:], in_=sm[:], func=mybir.ActivationFunctionType.Ln
            )
            lgb = lg[:].unsqueeze(2).to_broadcast([P, NT, NCLS])
            nc.vector.tensor_tensor(
                out=a3, in0=a3, in1=lgb, op=mybir.AluOpType.subtract
            )
            nc.sync.dma_start(out=out, in_=a2sb[:])

    nc.compile()
    return nc


def _rows(a):  # [P, NT*NCLS] sbuf layout -> [NR, NCLS] rank rows
    return np.ascontiguousarray(
        a.reshape(P, NT, NCLS).transpose(1, 0, 2).reshape(NR, NCLS)
    )


def _cols(rows):  # [NR, NCLS] rank rows -> [P, NT*NCLS] sbuf layout
    return np.ascontiguousarray(
        rows.reshape(NT, P, NCLS).transpose(1, 0, 2).reshape(P, NT * NCLS)
    )


def _zq_tables(z_full):
    """z_full [N_CORES*NR, NCLS] -> G2 padded tables [Q2+1, ZPAD]
    (last row zeros, the pad-slot target)."""
    out = []
    for g in range(G2):
        t = np.zeros((Q2 + 1, ZPAD), np.float32)
        t[:Q2, :NCLS] = z_full[g * Q2 : (g + 1) * Q2]
        out.append(t)
    return out


def kernel(
    x, edge_src, edge_dst, W_neigh1, W_self1, b1, W_neigh2, W_self2, b2
):
    x = np.ascontiguousarray(np.asarray(x, dtype=np.float32))
    orders1, orders2, grids1, grids2, xTs = _prep_host(x, edge_src, edge_dst)

    common = {
        **{f"xq{g}": np.vstack(
            [x[g * Q1 : (g + 1) * Q1], np.zeros((1, P), np.float32)])
           for g in range(G1)},
        "W1n": np.asarray(W_neigh1, np.float32),
        "W1s": np.asarray(W_self1, np.float32),
        "W2n": np.asarray(W_neigh2, np.float32),
        "W2s": np.asarray(W_self2, np.float32),
        "b1": np.asarray(b1, np.float32).reshape(1, P),
        "b2": np.asarray(b2, np.float32).reshape(1, NCLS),
        "Ident": np.eye(P, dtype=np.float32),
    }

    # NOTE: grids differ per core -> per-core programs would differ. SPMD
    # needs ONE program; use the max structure (see _unify_grids below).
    grid1 = _unify_grids(grids1)
    grid2 = _unify_grids(grids2)
    in_maps1 = [
        {**common, "xT": xTs[k], "I1": _pad_idx(grids1[k], grid1, Q1)}
        for k in range(N_CORES)
    ]
    nc1 = _build_nc1(grid1)
    res1 = run_bass_kernel_spmd(nc1, in_maps1, list(range(N_CORES)))

    z_full = np.concatenate(
        [_rows(res1.results[k]["z"]) for k in range(N_CORES)], axis=0
    )
    zqs = _zq_tables(z_full)
    in_maps2 = []
    for k in range(N_CORES):
        # re-lay o2 rows from L1 rank order into L2 rank order
        rank1l = np.empty(NPC, np.int64)
        rank1l[orders1[k]] = np.arange(NPC)
        perm = rank1l[orders2[k]]  # L2 rank -> L1 rank
        rows_o2 = _rows(res1.results[k]["o2"])
        o2_l2 = np.zeros((NR, NCLS), np.float32)
        o2_l2[:NPC] = rows_o2[perm]
        in_maps2.append(
            {
                **{f"zq{g}": zqs[g] for g in range(G2)},
                "o2": _cols(o2_l2),
                "I2": _pad_idx(grids2[k], grid2, Q2),
            }
        )
    nc2 = _build_nc2(grid2)
    res2 = run_bass_kernel_spmd(nc2, in_maps2, list(range(N_CORES)))

    out_full = np.empty((N_NODES, NCLS), dtype=np.float32)
    for k in range(N_CORES):
        out_full[k * NPC + orders2[k]] = _rows(res2.results[k]["out"])[:NPC]
    return out_full


def _unify_grids(grids):
    """All cores share one program: R = per-(t,g) max across cores."""
    R = np.maximum.reduce([g[1] for g in grids])
    col_base = np.zeros_like(R)
    col_base[:, 1:] = np.cumsum(R, axis=1)[:, :-1]
    W = R.sum(axis=1)
    tile_base = np.zeros(NT + 1, np.int64)
    tile_base[1:] = np.cumsum(W)
    return None, R, col_base, tile_base


def _pad_idx(core_grid, uni_grid, zrow):
    """Re-lay a core's slot matrix into the unified grid (pad rounds point
    at the zeros row), then wrap into the [128, TOTW*8] int16 idx layout."""
    A, Rc, cbc, tbc = core_grid
    _, Ru, cbu, tbu = uni_grid
    TOTW = int(tbu[-1])
    Au = np.full((TOTW, P), zrow, np.int16)
    for t in range(NT):
        for g in range(Ru.shape[1]):
            n = int(Rc[t, g])
            if n == 0:
                continue
            src0 = int(tbc[t]) + int(cbc[t, g])
            dst0 = int(tbu[t]) + int(cbu[t, g])
            Au[dst0 : dst0 + n] = A[src0 : src0 + n]
    return np.tile(Au.reshape(TOTW * 8, 16).T, (8, 1)).copy()


if __name__ == "__main__":
    import jax

    import reference

    cpu = jax.devices("cpu")[0]
    with jax.default_device(cpu):
        inputs = {k: np.asarray(v) for k, v in reference.setup_inputs().items()}
        exp = np.asarray(
            reference.reference(
                **{k: jax.device_put(v, cpu) for k, v in inputs.items()}
            )
        )
    got = kernel(**inputs)
    err = np.abs(got - exp)
    rel = err / (np.abs(exp) + 1e-6)
    print("max abs err:", err.max(), "max rel err:", rel.max())


# revision 3
# speedup vs baseline: 1.4669x; 1.1737x over previous
"""Trainium2 Bass kernel for a 2-layer GraphSAGE (sum aggregation) GNN — v2.

Strategy (8 NeuronCores, SPMD, two launches):
  - Nodes (dst) sharded 12500/core; dsts degree-sorted into 98 tiles of 128
    "ranks". Edges land in per-(tile, group) slot grids: slot (p, c) holds
    the c-th source of rank p, padded with -1 (dma_gather writes zeros for
    mid-list negative idxs — the additive identity).
  - Gathers use nc.gpsimd.dma_gather (int16 idxs, purpose-built Q7 ucode,
    load_library(mlp)) instead of per-round indirect_dma_start: many rounds
    per instruction instead of 1, cutting Pool-engine instruction count ~4-8x.
  - int16 idx range (<=32767) forces table splits: x into 4 quarters of
    25000 rows (layer 1); z into 4 owner-pair tables of 25088 rows padded to
    64 f32 = 256B rows (dma_gather min row size; layer 2).
  - Per tile: DVE fold (halving adds) reduces the gathered slots to the
    aggregated row, then PE: h = relu(aggT@W1n + xT@W1s + b1),
    z = h@W2n, o2 = h@W2s + b2. Only z (8 wide) crosses cores (host concat).
  - Launch 2: same grid structure gathers z rows (256B padded), strided
    8-wide folds, + o2, log_softmax.
"""

import sys

import numpy as np

sys.path.insert(0, "/opt/trn_rl_repo")

import concourse.bass as bass
import concourse.mybir as mybir
import concourse.tile as tile
from concourse import bacc, library_config
from concourse.bass_utils import run_bass_kernel_spmd

P = 128
N_NODES = 100000
N_CORES = 8
NPC = N_NODES // N_CORES  # 12500
NT = 98  # rank tiles per core
NR = NT * P  # 12544 rank slots per core
NCLS = 8
G1 = 4  # x table split (25000 rows each, int16-addressable)
Q1 = N_NODES // G1  # 25000
G2 = 4  # z table split (owner pairs: 25088 rows each)
Q2 = 2 * NR  # 25088
ZPAD = 64  # padded z row elems (256B = dma_gather minimum)
RC = 8  # rounds (128-row groups) per dma_gather instruction (<=1024 idxs)
PAD_NEG = False  # True: pad slots are -1 (ucode zero-fill, no HBM read);
# each instruction-chunk's final slot is patched to the zeros row so the
# trailing-negative skip can never leave stale data.
F32 = mybir.dt.float32
I16 = mybir.dt.int16


def _build_grid(r, g, v, G, zrow):
    """Slot grids for one core. r: dst rank/edge; g: table group/edge;
    v: local table row/edge; zrow: zeros-row index used for pad slots.
    Returns (A [TOTW, 128] int16 slot matrix, R [NT, G] rounds,
    col_base [NT, G], tile_base [NT+1])."""
    o = np.lexsort((r, g))
    gs, rs, vs = g[o], r[o], v[o]
    gr = gs.astype(np.int64) * NR + rs
    occ = np.arange(len(gr)) - np.searchsorted(gr, gr)
    ts = rs // P
    ps = rs % P
    R = np.zeros((NT, G), np.int64)
    np.maximum.at(R, (ts, gs), occ + 1)
    col_base = np.zeros((NT, G), np.int64)
    col_base[:, 1:] = np.cumsum(R, axis=1)[:, :-1]
    W = R.sum(axis=1)  # rounds per tile
    tile_base = np.zeros(NT + 1, np.int64)
    tile_base[1:] = np.cumsum(W)
    TOTW = int(tile_base[-1])
    A = np.full((TOTW, P), -1 if PAD_NEG else zrow, np.int16)
    gcol = tile_base[ts] + col_base[ts, gs] + occ
    A[gcol, ps] = vs.astype(np.int16)
    return A, R, col_base, tile_base


def _profile_order(dl, g, G):
    """Order local dst nodes so tiles hold similar per-group-count profiles
    (minimizes per-(tile,group) max-padding). Descending lexsort by profile."""
    cnt = np.zeros((NPC, G), np.int64)
    np.add.at(cnt, (dl, g), 1)
    order = np.lexsort(tuple(cnt[:, i] for i in range(G - 1, -1, -1)))[::-1]
    rank_of = np.empty(NPC, np.int64)
    rank_of[order] = np.arange(NPC)
    return order, rank_of


def _prep_host(x, edge_src, edge_dst):
    edge_src = np.asarray(edge_src).astype(np.int64)
    edge_dst = np.asarray(edge_dst).astype(np.int64)
    core_of = edge_dst // NPC
    S, D = [], []
    for k in range(N_CORES):
        m = core_of == k
        S.append(edge_src[m])
        D.append(edge_dst[m] - k * NPC)

    # pass A: layer-1 orders (by x-quarter profile) -> zpos layout
    orders1, grids1 = [], []
    rank1_of = np.empty(N_NODES, np.int64)
    for k in range(N_CORES):
        order1, r1 = _profile_order(D[k], S[k] // Q1, G1)
        orders1.append(order1)
        rank1_of[k * NPC : (k + 1) * NPC] = r1
        grids1.append(_build_grid(r1[D[k]], S[k] // Q1, S[k] % Q1, G1, Q1))
    owner = np.arange(N_NODES) // NPC
    zpos = owner * NR + rank1_of

    # pass B: layer-2 orders (by z-group profile)
    orders2, grids2 = [], []
    for k in range(N_CORES):
        zp = zpos[S[k]]
        order2, r2 = _profile_order(D[k], zp // Q2, G2)
        orders2.append(order2)
        grids2.append(_build_grid(r2[D[k]], zp // Q2, zp % Q2, G2, Q2))

    xTs = []
    x = np.ascontiguousarray(np.asarray(x, dtype=np.float32))
    for k in range(N_CORES):
        xT = np.zeros((P, NR), np.float32)
        xT[:, :NPC] = x[k * NPC + orders1[k]].T
        xTs.append(np.ascontiguousarray(xT))

    return orders1, orders2, grids1, grids2, xTs


def _chunks(n):
    """Split n rounds into chunks of <= RC."""
    out = []
    a = 0
    while a < n:
        b = min(a + RC, n)
        out.append((a, b))
        a = b
    return out


def _emit_gathers(nc, grid, tabs, it, buf, t, elem):
    """Emit dma_gather instructions for tile t into buf; returns rounds W."""
    _, R, col_base, tile_base = grid
    W = int(tile_base[t + 1] - tile_base[t])
    for g in range(len(tabs)):
        Rg = int(R[t, g])
        if Rg == 0:
            continue
        lc = int(col_base[t, g])  # local col (rounds) within tile
        gc = int(tile_base[t]) + lc  # global col
        for a, b in _chunks(Rg):
            nb = b - a
            nc.gpsimd.dma_gather(
                out_ap=buf[:, (lc + a) * elem : (lc + b) * elem].rearrange(
                    "p (c e) -> p c e", e=elem
                ),
                in_ap=tabs[g],
                idxs_ap=it[:, (gc + a) * 8 : (gc + b) * 8],
                num_idxs=nb * P,
                num_idxs_reg=nb * P,
                elem_size=elem,
                single_packet=True,
            )
    return W


def _build_nc1(grid1, reps=1):
    """Launch 1: layer-1 aggregate + matmuls; outputs z and o2."""
    idx1, R1, col_base1, tile_base1 = grid1
    TOTW = int(tile_base1[-1])
    WMAX = int((tile_base1[1:] - tile_base1[:-1]).max())
    nc = bacc.Bacc(
        "TRN2", target_bir_lowering=False, debug=False, num_devices=N_CORES
    )
    xq = [
        nc.dram_tensor(f"xq{g}", [Q1 + 1, P], F32, kind="ExternalInput").ap()
        for g in range(G1)
    ]
    xT = nc.dram_tensor("xT", [P, NR], F32, kind="ExternalInput").ap()
    I1 = nc.dram_tensor("I1", [P, TOTW * 8], I16, kind="ExternalInput").ap()
    W1n = nc.dram_tensor("W1n", [P, P], F32, kind="ExternalInput").ap()
    W1s = nc.dram_tensor("W1s", [P, P], F32, kind="ExternalInput").ap()
    W2n = nc.dram_tensor("W2n", [P, NCLS], F32, kind="ExternalInput").ap()
    W2s = nc.dram_tensor("W2s", [P, NCLS], F32, kind="ExternalInput").ap()
    b1 = nc.dram_tensor("b1", [1, P], F32, kind="ExternalInput").ap()
    b2 = nc.dram_tensor("b2", [1, NCLS], F32, kind="ExternalInput").ap()
    Ident = nc.dram_tensor("Ident", [P, P], F32, kind="ExternalInput").ap()
    z_k = nc.dram_tensor("z", [P, NT * NCLS], F32, kind="ExternalOutput").ap()
    o2_k = nc.dram_tensor("o2", [P, NT * NCLS], F32, kind="ExternalOutput").ap()

    with tile.TileContext(nc) as tc:
        with (
            tc.tile_pool(name="persist", bufs=1) as pp,
            tc.tile_pool(name="gather", bufs=3) as gp,
            tc.tile_pool(name="work", bufs=3) as wp,
            tc.tile_pool(name="psum", bufs=1, space="PSUM") as psp,
        ):
            w1n = pp.tile([P, P], F32, tag="w1n")
            w1s = pp.tile([P, P], F32, tag="w1s")
            w2n = pp.tile([P, NCLS], F32, tag="w2n")
            w2s = pp.tile([P, NCLS], F32, tag="w2s")
            b1t = pp.tile([1, P], F32, tag="b1")
            b2t = pp.tile([1, NCLS], F32, tag="b2")
            ones = pp.tile([1, P], F32, tag="ones")
            ident = pp.tile([P, P], F32, tag="ident")
            i1t = pp.tile([P, TOTW * 8], I16, tag="i1")
            xTt = pp.tile([P, NR], F32, tag="xT")
            zsb = pp.tile([P, NT * NCLS], F32, tag="z")
            o2sb = pp.tile([P, NT * NCLS], F32, tag="o2")

            nc.gpsimd.load_library(library_config.mlp)
            nc.sync.dma_start(out=w1n[:], in_=W1n[:])
            nc.sync.dma_start(out=w1s[:], in_=W1s[:])
            nc.sync.dma_start(out=w2n[:], in_=W2n[:])
            nc.sync.dma_start(out=w2s[:], in_=W2s[:])
            nc.sync.dma_start(out=b1t[:], in_=b1[:])
            nc.sync.dma_start(out=b2t[:], in_=b2[:])
            nc.sync.dma_start(out=i1t[:], in_=I1[:])
            nc.sync.dma_start(out=xTt[:], in_=xT[:])
            nc.sync.dma_start(out=ident[:], in_=Ident[:])
            nc.vector.memset(ones[:], 1.0)

            for rep in range(reps):
                for t in range(NT):
                    csl = slice(t * P, (t + 1) * P)
                    zsl = slice(t * NCLS, (t + 1) * NCLS)
                    buf = gp.tile([P, WMAX * P], F32, tag="g1")
                    W = _emit_gathers(
                        nc, grid1, xq, i1t[:], buf[:], t, P
                    )
                    if W == 0:
                        nc.vector.memset(buf[:, :P], 0.0)
                    w = W
                    while w > 1:
                        h = w // 2
                        nc.vector.tensor_add(
                            out=buf[:, : h * P],
                            in0=buf[:, : h * P],
                            in1=buf[:, (w - h) * P : w * P],
                        )
                        w -= h
                    aggT_ps = psp.tile([P, P], F32, tag="aggT_ps")
                    nc.tensor.transpose(
                        out=aggT_ps[:], in_=buf[:, :P], identity=ident[:]
                    )
                    aggT = wp.tile([P, P], F32, tag="aggT")
                    nc.vector.tensor_copy(out=aggT[:], in_=aggT_ps[:])
                    h_ps = psp.tile([P, P], F32, tag="h_ps")
                    nc.tensor.matmul(
                        out=h_ps[:], lhsT=aggT[:], rhs=w1n[:],
                        start=True, stop=False,
                    )
                    nc.tensor.matmul(
                        out=h_ps[:], lhsT=xTt[:, csl], rhs=w1s[:],
                        start=False, stop=False,
                    )
                    nc.tensor.matmul(
                        out=h_ps[:], lhsT=ones[:1, :], rhs=b1t[:1, :],
                        start=False, stop=True,
                    )
                    hsb = wp.tile([P, P], F32, tag="h")
                    nc.scalar.activation(
                        out=hsb[:], in_=h_ps[:],
                        func=mybir.ActivationFunctionType.Relu,
                    )
                    hT_ps = psp.tile([P, P], F32, tag="hT_ps")
                    nc.tensor.transpose(
                        out=hT_ps[:], in_=hsb[:], identity=ident[:]
                    )
                    hT = wp.tile([P, P], F32, tag="hT")
                    nc.vector.tensor_copy(out=hT[:], in_=hT_ps[:])
                    z_ps = psp.tile([P, NCLS], F32, tag="z_ps")
                    nc.tensor.matmul(
                        out=z_ps[:], lhsT=hT[:], rhs=w2n[:],
                        start=True, stop=True,
                    )
                    nc.vector.tensor_copy(out=zsb[:, zsl], in_=z_ps[:])
                    o2_ps = psp.tile([P, NCLS], F32, tag="o2_ps")
                    nc.tensor.matmul(
                        out=o2_ps[:], lhsT=hT[:], rhs=w2s[:],
                        start=True, stop=False,
                    )
                    nc.tensor.matmul(
                        out=o2_ps[:], lhsT=ones[:1, :], rhs=b2t[:1, :],
                        start=False, stop=True,
                    )
                    nc.vector.tensor_copy(out=o2sb[:, zsl], in_=o2_ps[:])

            nc.sync.dma_start(out=z_k, in_=zsb[:])
            nc.sync.dma_start(out=o2_k, in_=o2sb[:])

    nc.compile()
    return nc


def _build_nc2(grid2, reps=1):
    """Launch 2: layer-2 gather of padded z rows, + self path, log_softmax."""
    idx2, R2, col_base2, tile_base2 = grid2
    TOTW = int(tile_base2[-1])
    WMAX = int((tile_base2[1:] - tile_base2[:-1]).max())
    nc = bacc.Bacc(
        "TRN2", target_bir_lowering=False, debug=False, num_devices=N_CORES
    )
    zq = [
        nc.dram_tensor(f"zq{g}", [Q2 + 1, ZPAD], F32, kind="ExternalInput").ap()
        for g in range(G2)
    ]
    o2_k = nc.dram_tensor("o2", [P, NT * NCLS], F32, kind="ExternalInput").ap()
    I2 = nc.dram_tensor("I2", [P, TOTW * 8], I16, kind="ExternalInput").ap()
    out = nc.dram_tensor("out", [P, NT * NCLS], F32, kind="ExternalOutput").ap()

    with tile.TileContext(nc) as tc:
        with (
            tc.tile_pool(name="persist", bufs=1) as pp,
            tc.tile_pool(name="gather", bufs=4) as gp,
        ):
            i2t = pp.tile([P, TOTW * 8], I16, tag="i2")
            o2sb = pp.tile([P, NT * NCLS], F32, tag="o2")
            a2sb = pp.tile([P, NT * NCLS], F32, tag="a2")
            nc.gpsimd.load_library(library_config.mlp)
            nc.sync.dma_start(out=i2t[:], in_=I2[:])
            nc.sync.dma_start(out=o2sb[:], in_=o2_k[:])

            for rep in range(reps):
                for t in range(NT):
                    zsl = slice(t * NCLS, (t + 1) * NCLS)
                    buf = gp.tile([P, WMAX * ZPAD], F32, tag="g2")
                    W = _emit_gathers(
                        nc, grid2, zq, i2t[:], buf[:], t, ZPAD
                    )
                    if W == 0:
                        nc.vector.memset(buf[:, :ZPAD], 0.0)
                    v3 = buf[:].rearrange("p (w e) -> p w e", e=ZPAD)
                    w = W
                    while w > 1:
                        h = w // 2
                        nc.vector.tensor_tensor(
                            out=v3[:, :h, :NCLS],
                            in0=v3[:, :h, :NCLS],
                            in1=v3[:, w - h : w, :NCLS],
                            op=mybir.AluOpType.add,
                        )
                        w -= h
                    nc.vector.tensor_add(
                        out=a2sb[:, zsl], in0=buf[:, :NCLS], in1=o2sb[:, zsl]
                    )

            a3 = a2sb[:].rearrange("p (t c) -> p t c", c=NCLS)
            mx = pp.tile([P, NT], F32, tag="mx")
            nc.vector.tensor_reduce(
                out=mx[:], in_=a3, axis=mybir.AxisListType.X,
                op=mybir.AluOpType.max,
            )
            mxb = mx[:].unsqueeze(2).to_broadcast([P, NT, NCLS])
            nc.vector.tensor_tensor(
                out=a3, in0=a3, in1=mxb, op=mybir.AluOpType.subtract
            )
            ex = pp.tile([P, NT * NCLS], F32, tag="ex")
            nc.scalar.activation(
                out=ex[:], in_=a2sb[:], func=mybir.ActivationFunctionType.Exp
            )
            sm = pp.tile([P, NT], F32, tag="sm")
            nc.vector.tensor_reduce(
                out=sm[:],
                in_=ex[:].rearrange("p (t c) -> p t c", c=NCLS),
                axis=mybir.AxisListType.X,
                op=mybir.AluOpType.add,
            )
            lg = pp.tile([P, NT], F32, tag="lg")
            nc.scalar.activation(
                out=lg[:], in_=sm[:], func=mybir.ActivationFunctionType.Ln
            )
            lgb = lg[:].unsqueeze(2).to_broadcast([P, NT, NCLS])
            nc.vector.tensor_tensor(
                out=a3, in0=a3, in1=lgb, op=mybir.AluOpType.subtract
            )
            nc.sync.dma_start(out=out, in_=a2sb[:])

    nc.compile()
    return nc


def _rows(a):  # [P, NT*NCLS] sbuf layout -> [NR, NCLS] rank rows
    return np.ascontiguousarray(
        a.reshape(P, NT, NCLS).transpose(1, 0, 2).reshape(NR, NCLS)
    )


def _cols(rows):  # [NR, NCLS] rank rows -> [P, NT*NCLS] sbuf layout
    return np.ascontiguousarray(
        rows.reshape(NT, P, NCLS).transpose(1, 0, 2).reshape(P, NT * NCLS)
    )


def _zq_tables(z_full):
    """z_full [N_CORES*NR, NCLS] -> G2 padded tables [Q2+1, ZPAD]
    (last row zeros, the pad-slot target)."""
    out = []
    for g in range(G2):
        t = np.zeros((Q2 + 1, ZPAD), np.float32)
        t[:Q2, :NCLS] = z_full[g * Q2 : (g + 1) * Q2]
        out.append(t)
    return out


def kernel(
    x, edge_src, edge_dst, W_neigh1, W_self1, b1, W_neigh2, W_self2, b2
):
    x = np.ascontiguousarray(np.asarray(x, dtype=np.float32))
    orders1, orders2, grids1, grids2, xTs = _prep_host(x, edge_src, edge_dst)

    common = {
        **{f"xq{g}": np.vstack(
            [x[g * Q1 : (g + 1) * Q1], np.zeros((1, P), np.float32)])
           for g in range(G1)},
        "W1n": np.asarray(W_neigh1, np.float32),
        "W1s": np.asarray(W_self1, np.float32),
        "W2n": np.asarray(W_neigh2, np.float32),
        "W2s": np.asarray(W_self2, np.float32),
        "b1": np.asarray(b1, np.float32).reshape(1, P),
        "b2": np.asarray(b2, np.float32).reshape(1, NCLS),
        "Ident": np.eye(P, dtype=np.float32),
    }

    # NOTE: grids differ per core -> per-core programs would differ. SPMD
    # needs ONE program; use the max structure (see _unify_grids below).
    grid1 = _unify_grids(grids1)
    grid2 = _unify_grids(grids2)
    in_maps1 = [
        {**common, "xT": xTs[k], "I1": _pad_idx(grids1[k], grid1, Q1)}
        for k in range(N_CORES)
    ]
    nc1 = _build_nc1(grid1)
    res1 = run_bass_kernel_spmd(nc1, in_maps1, list(range(N_CORES)))

    z_full = np.concatenate(
        [_rows(res1.results[k]["z"]) for k in range(N_CORES)], axis=0
    )
    zqs = _zq_tables(z_full)
    in_maps2 = []
    for k in range(N_CORES):
        # re-lay o2 rows from L1 rank order into L2 rank order
        rank1l = np.empty(NPC, np.int64)
        rank1l[orders1[k]] = np.arange(NPC)
        perm = rank1l[orders2[k]]  # L2 rank -> L1 rank
        rows_o2 = _rows(res1.results[k]["o2"])
        o2_l2 = np.zeros((NR, NCLS), np.float32)
        o2_l2[:NPC] = rows_o2[perm]
        in_maps2.append(
            {
                **{f"zq{g}": zqs[g] for g in range(G2)},
                "o2": _cols(o2_l2),
                "I2": _pad_idx(grids2[k], grid2, Q2),
            }
        )
    nc2 = _build_nc2(grid2)
    res2 = run_bass_kernel_spmd(nc2, in_maps2, list(range(N_CORES)))

    out_full = np.empty((N_NODES, NCLS), dtype=np.float32)
    for k in range(N_CORES):
        out_full[k * NPC + orders2[k]] = _rows(res2.results[k]["out"])[:NPC]
    return out_full


def _unify_grids(grids):
    """All cores share one program: R = per-(t,g) max across cores."""
    R = np.maximum.reduce([g[1] for g in grids])
    col_base = np.zeros_like(R)
    col_base[:, 1:] = np.cumsum(R, axis=1)[:, :-1]
    W = R.sum(axis=1)
    tile_base = np.zeros(NT + 1, np.int64)
    tile_base[1:] = np.cumsum(W)
    return None, R, col_base, tile_base


def _pad_idx(core_grid, uni_grid, zrow):
    """Re-lay a core's slot matrix into the unified grid (pad slots point
    at the zeros row, or -1 when PAD_NEG), then wrap into the
    [128, TOTW*8] int16 idx layout."""
    A, Rc, cbc, tbc = core_grid
    _, Ru, cbu, tbu = uni_grid
    TOTW = int(tbu[-1])
    fill = -1 if PAD_NEG else zrow
    Au = np.full((TOTW, P), fill, np.int16)
    for t in range(NT):
        for g in range(Ru.shape[1]):
            n = int(Rc[t, g])
            if n == 0:
                continue
            src0 = int(tbc[t]) + int(cbc[t, g])
            dst0 = int(tbu[t]) + int(cbu[t, g])
            Au[dst0 : dst0 + n] = A[src0 : src0 + n]
    if PAD_NEG:
        # a trailing -1 run in an instruction's flat idx list is skipped
        # (no write) -> final flat slot of every chunk must be valid
        for t in range(NT):
            for g in range(Ru.shape[1]):
                base = int(tbu[t]) + int(cbu[t, g])
                for a, b in _chunks(int(Ru[t, g])):
                    if Au[base + b - 1, P - 1] < 0:
                        Au[base + b - 1, P - 1] = zrow
    return np.tile(Au.reshape(TOTW * 8, 16).T, (8, 1)).copy()


if __name__ == "__main__":
    import jax

    import reference

    cpu = jax.devices("cpu")[0]
    with jax.default_device(cpu):
        inputs = {k: np.asarray(v) for k, v in reference.setup_inputs().items()}
        exp = np.asarray(
            reference.reference(
                **{k: jax.device_put(v, cpu) for k, v in inputs.items()}
            )
        )
    got = kernel(**inputs)
    err = np.abs(got - exp)
    rel = err / (np.abs(exp) + 1e-6)
    print("max abs err:", err.max(), "max rel err:", rel.max())
